# revision 23
# baseline (speedup 1.0000x reference)
"""GCN (2-layer, PyG GCNConv semantics) on 8 Trainium2 NeuronCores.

Strategy
--------
All device work is DENSE segment-sums over host-built, dst-sorted edge
grids (per-edge gather/scatter on TRN2 is far too slow). The kernel
stays near the HBM roofline by:

1. No per-slot normalization math. A tiny NEFF0 computes per-NODE
   u = dinv*x and dinv (dinv = 1/sqrt(deg+1)); the host gathers the
   already-normalized per-node fp16 values into the edge grids, so the
   big NEFFs only do segment sums. Self-loops are materialized as one
   extra slot per node, so no separate self-term pass is needed.
2. fp16 grids: halves HBM traffic (fp16 values are produced ON DEVICE;
   the host only moves bytes).
3. Degree-sorted rows with per-chunk padding: nodes are sorted by
   degree (desc) inside each core, rows take 128 consecutive ranks, and
   each chunk of rows is padded to its own max degree -> padding
   inflation ~1.2x instead of maxdeg/meandeg ~2.5x.
4. Segment sums via pairwise fold adds on a SLOT-MAJOR grid layout
   ([slot, chan, row] per chunk): wide contiguous fp16 tensor_tensor
   adds, halving the slot count per step (odd counts defer one carry
   leaf), beat TENSOR_REDUCE over short padded rows (which pays a
   ~20-cycle restart per row). A slice of early-middle chunks goes to
   the (slower) GpSimd engine to overlap with the vector engine.

Math (A = D^-1/2 (Adj+I) D^-1/2, deg counts in-edges at dst +1):
  y1[v]   = dinv[v]*sum_{e->v, incl self} u[src],  u = dinv*x
  M[v,c]  = dinv[v]*relu(W1[0,c]*y1[v] + b1[c])
  z[v,c]  = dinv[v]*sum_{e->v, incl self} M[src,c]
  out     = z @ W2 + b2

NEFF0: per-node u, dinv.  NEFF1: layer-1 segment sums + M.  NEFF2:
4-channel segment sums + W2 combine. Host work between launches is
pure index work (sort/gather/scatter/pad of device-produced bytes).
"""
import math
import sys

sys.path.insert(0, "/opt/trn_rl_repo")

import numpy as np

N_NODES = 500_000
N_EDGES = 16_000_000
N_CORES = 8
NPC = N_NODES // N_CORES        # 62500 nodes per core
NROWPP = 496                    # rows per partition (128*496 = 63488 >= NPC)
NROWTOT = 128 * NROWPP
NCHUNK1 = 4                     # layer-1 grid: 4 chunks x 124 rows
CROWS1 = NROWPP // NCHUNK1
NCHUNK2 = 16                    # layer-2 grid: 16 chunks x 31 rows
CROWS2 = NROWPP // NCHUNK2

_NEFF_CACHE: dict = {}


def _fold_chunk(nc, pool, f16, gt, pad, unit, out_ap, tagp, eng=None):
    """Segment-sum a slot-major chunk tile gt [128, pad*unit] (slot index
    outer) into f32 out_ap [128, unit] via non-in-place halving fp16 adds
    (ping-pong buffers); odd slot-counts defer one carry leaf. `eng`
    selects the engine (vector default; gpsimd to offload)."""
    if eng is None:
        eng = nc.vector
    halfw = max(unit, (pad // 2) * unit)
    fa = pool.tile([128, halfw], f16, tag=tagp + "a")
    fb = pool.tile([128, max(unit, halfw // 2)], f16, tag=tagp + "b")
    cur, s = gt, pad
    nxt = fa
    carries = []
    while s > 2:
        m = s // 2
        if s % 2:
            carries.append(cur[:, 2 * m * unit:s * unit])
        eng.tensor_add(out=nxt[:, :m * unit], in0=cur[:, :m * unit],
                       in1=cur[:, m * unit:2 * m * unit])
        cur, nxt = nxt, (fb if nxt is fa else fa)
        s = m
    leaves = [cur[:, i * unit:(i + 1) * unit] for i in range(s)] + carries
    while len(leaves) > 2:
        t = pool.tile([128, unit], f16, tag=tagp + "c")
        eng.tensor_add(out=t[:], in0=leaves[0], in1=leaves[1])
        leaves = [t[:]] + leaves[2:]

    def _shp(ap2d):
        if len(out_ap.shape) == 3:
            return ap2d.rearrange("p (c r) -> p c r", r=out_ap.shape[-1])
        return ap2d

    if len(leaves) == 2:
        eng.tensor_add(out=out_ap, in0=_shp(leaves[0]), in1=_shp(leaves[1]))
    else:
        eng.tensor_copy(out=out_ap, in_=_shp(leaves[0]))


def _build_neff0():
    """Per-node: dinv = 1/sqrt(deg+1); u = x*dinv (fp16 out)."""
    from concourse import bacc, mybir, tile

    nc = bacc.Bacc("TRN2", target_bir_lowering=False, debug=False,
                   num_devices=N_CORES)
    f32, f16, u16 = mybir.dt.float32, mybir.dt.float16, mybir.dt.uint16
    xo = nc.dram_tensor("xo", [128, NROWPP], f32, kind="ExternalInput")
    dg = nc.dram_tensor("dg", [128, NROWPP], u16, kind="ExternalInput")
    uo = nc.dram_tensor("uo", [128, NROWPP], f16, kind="ExternalOutput")
    dv = nc.dram_tensor("dv", [128, NROWPP], f32, kind="ExternalOutput")

    with tile.TileContext(nc) as tc:
        with tc.tile_pool(name="p", bufs=1) as pool:
            sh = [128, NROWPP]
            xt = pool.tile(sh, f32, tag="x")
            dt_ = pool.tile(sh, u16, tag="d")
            nc.sync.dma_start(out=xt[:], in_=xo.ap())
            nc.sync.dma_start(out=dt_[:], in_=dg.ap())
            df = pool.tile(sh, f32, tag="df")
            nc.vector.tensor_copy(out=df[:], in_=dt_[:])
            rc = pool.tile(sh, f32, tag="rc")
            nc.vector.reciprocal_approx_fast(out=rc[:], in_=df[:])
            dvt = pool.tile(sh, f32, tag="dv")
            nc.scalar.sqrt(out=dvt[:], in_=rc[:])
            ut = pool.tile(sh, f16, tag="u")
            nc.vector.tensor_tensor(out=ut[:], in0=xt[:], in1=dvt[:],
                                    op=mybir.AluOpType.mult)
            nc.sync.dma_start(out=uo.ap(), in_=ut[:])
            nc.sync.dma_start(out=dv.ap(), in_=dvt[:])
    nc.compile()
    return nc


def _build_neff1(pads1):
    """Layer 1: y1 = dinv*segsum(u[src] incl self);
    M[v,c] = dinv*relu(W1c*y1+b1c) -> fp16 planes [128, 4*NROWPP]."""
    from concourse import bacc, mybir, tile

    nc = bacc.Bacc("TRN2", target_bir_lowering=False, debug=False,
                   num_devices=N_CORES)
    f32, f16 = mybir.dt.float32, mybir.dt.float16
    mult = mybir.AluOpType.mult
    Relu = mybir.ActivationFunctionType.Relu
    totw = sum(CROWS1 * p for p in pads1)
    padmax = max(pads1)

    gu = nc.dram_tensor("gu", [128, totw], f16, kind="ExternalInput")
    dv = nc.dram_tensor("dv", [128, NROWPP], f32, kind="ExternalInput")
    w1r = nc.dram_tensor("w1r", [128, 4], f32, kind="ExternalInput")
    b1r = nc.dram_tensor("b1r", [128, 4], f32, kind="ExternalInput")
    mo = nc.dram_tensor("mo", [128, 4 * NROWPP], f16, kind="ExternalOutput")

    with tile.TileContext(nc) as tc:
        with tc.tile_pool(name="p", bufs=4) as pool, \
             tc.tile_pool(name="t", bufs=2) as tpool, \
             tc.tile_pool(name="s", bufs=1) as spool:
            seg = spool.tile([128, NROWPP], f32)
            off = 0
            for k, pad in enumerate(pads1):
                w = CROWS1 * pad
                gt = pool.tile([128, CROWS1 * padmax], f16, tag="g")
                nc.sync.dma_start(out=gt[:, :w], in_=gu.ap()[:, off:off + w])
                _fold_chunk(nc, tpool, f16, gt, pad, CROWS1,
                            seg[:, k * CROWS1:(k + 1) * CROWS1], "fv")
                off += w
            dvt = spool.tile([128, NROWPP], f32, tag="dv")
            w1t = spool.tile([128, 4], f32, tag="w1")
            b1t = spool.tile([128, 4], f32, tag="b1")
            nc.sync.dma_start(out=dvt[:], in_=dv.ap())
            nc.sync.dma_start(out=w1t[:], in_=w1r.ap())
            nc.sync.dma_start(out=b1t[:], in_=b1r.ap())
            # hoist the Relu activation-table load off the finalize path
            warm = spool.tile([128, 4], f32, tag="warm")
            nc.scalar.activation(out=warm[:], in_=w1t[:], func=Relu)
            nc.vector.tensor_tensor(out=seg[:], in0=seg[:], in1=dvt[:],
                                    op=mult)
            for c in range(4):
                h = tpool.tile([128, NROWPP], f32, tag="h")
                nc.scalar.activation(out=h[:], in_=seg[:], func=Relu,
                                     bias=b1t[:, c:c + 1],
                                     scale=w1t[:, c:c + 1])
                eng1 = nc.vector if c % 2 == 0 else nc.gpsimd
                m16 = tpool.tile([128, NROWPP], f16, tag=f"m16{c % 2}")
                eng1.tensor_tensor(out=m16[:], in0=h[:], in1=dvt[:], op=mult)
                nc.sync.dma_start(
                    out=mo.ap()[:, c * NROWPP:(c + 1) * NROWPP], in_=m16[:])
    nc.compile()
    return nc


def _build_neff2(pads2):
    """Layer 2: z[v,c] = dinv*segsum(M[src,c] incl self); out = z@W2+b2
    (out planar [128, 4*NROWPP], channel-major)."""
    from concourse import bacc, mybir, tile

    nc = bacc.Bacc("TRN2", target_bir_lowering=False, debug=False,
                   num_devices=N_CORES)
    f32, f16 = mybir.dt.float32, mybir.dt.float16
    mult, add = mybir.AluOpType.mult, mybir.AluOpType.add
    Copy = mybir.ActivationFunctionType.Copy
    Ident = mybir.ActivationFunctionType.Identity
    totw = sum(CROWS2 * p for p in pads2)
    padmax = max(pads2)
    UNIT = 4 * CROWS2

    gm = nc.dram_tensor("gm", [128, 4 * totw], f16, kind="ExternalInput")
    dv = nc.dram_tensor("dv", [128, NROWPP], f32, kind="ExternalInput")
    w2r = nc.dram_tensor("w2r", [128, 16], f32, kind="ExternalInput")
    b2r = nc.dram_tensor("b2r", [128, 4], f32, kind="ExternalInput")
    out = nc.dram_tensor("out", [128, 4 * NROWPP], f32, kind="ExternalOutput")

    with tile.TileContext(nc) as tc:
        with tc.tile_pool(name="p", bufs=5) as pool, \
             tc.tile_pool(name="t", bufs=2) as tpool, \
             tc.tile_pool(name="q", bufs=2) as psm, \
             tc.tile_pool(name="s", bufs=1) as spool:
            # channel-planar sums S[p, (c k r)]; tails write strided views
            S = spool.tile([128, 4 * NROWPP], f32, tag="sp")
            s4 = S[:].rearrange("p (c k r) -> p c k r", c=4, r=CROWS2)
            off = 0
            for k, pad in enumerate(pads2):
                w = UNIT * pad
                gt = pool.tile([128, UNIT * padmax], f16, tag="g")
                nc.sync.dma_start(out=gt[:, :w], in_=gm.ap()[:, off:off + w])
                _fold_chunk(nc, tpool, f16, gt, pad, UNIT,
                            s4[:, :, k, :], "fv")
                off += w
            dvt = spool.tile([128, NROWPP], f32, tag="dv")
            w2t = spool.tile([128, 16], f32, tag="w2")
            b2t = spool.tile([128, 4], f32, tag="b2")
            nc.sync.dma_start(out=dvt[:], in_=dv.ap())
            nc.sync.dma_start(out=w2t[:], in_=w2r.ap())
            nc.sync.dma_start(out=b2t[:], in_=b2r.ap())
            # hoist the Copy/Identity activation-table load
            warm = psm.tile([128, 16], f32, tag="warm")
            nc.scalar.activation(out=warm[:], in_=w2t[:], func=Ident)
            # z = dinv*S
            for c in range(4):
                Sc = S[:, c * NROWPP:(c + 1) * NROWPP]
                nc.vector.tensor_tensor(out=Sc, in0=Sc, in1=dvt[:], op=mult)
            for j in range(4):
                # first term carries the bias: acc = w2[0,j]*z0 + b2[j]
                acc = psm.tile([128, NROWPP], f32, tag="acc")
                nc.scalar.activation(out=acc[:], in_=S[:, 0:NROWPP],
                                     func=Ident, bias=b2t[:, j:j + 1],
                                     scale=w2t[:, j:j + 1])
                for c in (1, 2):
                    nc.vector.scalar_tensor_tensor(
                        out=acc[:], in0=S[:, c * NROWPP:(c + 1) * NROWPP],
                        scalar=w2t[:, c * 4 + j:c * 4 + j + 1], in1=acc[:],
                        op0=mult, op1=add)
                oj = psm.tile([128, NROWPP], f32, tag="oj")
                nc.vector.scalar_tensor_tensor(
                    out=oj[:], in0=S[:, 3 * NROWPP:4 * NROWPP],
                    scalar=w2t[:, 12 + j:13 + j], in1=acc[:],
                    op0=mult, op1=add)
                nc.sync.dma_start(
                    out=out.ap()[:, j * NROWPP:(j + 1) * NROWPP], in_=oj[:])
    nc.compile()
    return nc


def _get_neffs(pads1, pads2):
    key = (tuple(pads1), tuple(pads2))
    if key not in _NEFF_CACHE:
        _NEFF_CACHE[key] = (_build_neff0(), _build_neff1(pads1),
                            _build_neff2(pads2))
    return _NEFF_CACHE[key]


def kernel(x, edge_index, W1, b1, W2, b2):
    from concourse import bass_utils

    x = np.asarray(x, dtype=np.float32)
    W1 = np.asarray(W1, dtype=np.float32)
    b1 = np.asarray(b1, dtype=np.float32)
    W2 = np.asarray(W2, dtype=np.float32)
    b2 = np.asarray(b2, dtype=np.float32)
    ei = np.asarray(edge_index)
    assert x.shape == (N_NODES, 1) and ei.shape == (2, N_EDGES)
    xf = np.ascontiguousarray(x.reshape(-1))
    src = ei[0].astype(np.int64)
    dst = ei[1].astype(np.int64)

    # ---- host layout (index work only) ----
    deg = np.bincount(dst, minlength=N_NODES)           # int64, no self loop
    degp1 = (deg + 1).astype(np.uint16)

    # per-core degree sort (desc, stable); rank s -> (p = s%128, r = s//128)
    deg2 = deg.reshape(N_CORES, NPC)
    order = np.argsort(-deg2, axis=1, kind="stable")    # [8, NPC] local ids
    sorted_ids = order + (np.arange(N_CORES)[:, None] * NPC)  # global ids
    srank = np.empty(N_NODES, np.int64)
    srank[sorted_ids.reshape(-1)] = np.tile(np.arange(NPC), N_CORES)

    dsorted = np.take_along_axis(deg2, order, axis=1)   # [8, NPC] descending
    dpad = np.zeros((N_CORES, NROWTOT), np.int64)
    dpad[:, :NPC] = dsorted
    rowmax = dpad.reshape(N_CORES, NROWPP, 128)[:, :, 0].max(axis=0)

    def mk_pads(nchunk, crows):
        # +1 slot per row for the self-loop edge
        return [max(8, int(math.ceil((int(rowmax[k * crows]) + 1) / 8) * 8))
                for k in range(nchunk)]

    pads1 = mk_pads(NCHUNK1, CROWS1)
    pads2 = mk_pads(NCHUNK2, CROWS2)
    totw1 = sum(CROWS1 * p for p in pads1)
    totw2 = sum(CROWS2 * p for p in pads2)

    # per-row chunk-base offsets; slot j adds j*CROWS (grid1: [s, r] layout)
    # or j*4*CROWS + c*CROWS (grid2: [s, c, r] layout)
    roff1 = np.empty(NROWPP, np.int64)
    roff2 = np.empty(NROWPP, np.int64)
    off = 0
    for k, pad in enumerate(pads1):
        r0 = k * CROWS1
        roff1[r0:r0 + CROWS1] = off + np.arange(CROWS1, dtype=np.int64)
        off += CROWS1 * pad
    off = 0
    for k, pad in enumerate(pads2):
        r0 = k * CROWS2
        roff2[r0:r0 + CROWS2] = off + np.arange(CROWS2, dtype=np.int64)
        off += 4 * CROWS2 * pad

    # edge -> (core, partition, row, slot); self-loop gets slot deg[v]
    key = (dst << 19) | src                             # N_NODES < 2**19
    key.sort(kind="stable")
    sdst = key >> 19
    ssrc = (key & 0x7FFFF).astype(np.int64)
    ptr = np.zeros(N_NODES + 1, np.int64)
    np.cumsum(deg, out=ptr[1:])
    jslot = np.arange(N_EDGES, dtype=np.int64) - ptr[sdst]
    corei = sdst // NPC
    s_e = srank[sdst]
    p_e = s_e & 127
    r_e = s_e >> 7
    flat1 = p_e * totw1 + roff1[r_e] + jslot * CROWS1
    # self-loop slots (one per node)
    allv = np.arange(N_NODES, dtype=np.int64)
    s_v = srank[allv]
    p_v = s_v & 127
    r_v = s_v >> 7
    core_v = allv // NPC
    flat1_self = p_v * totw1 + roff1[r_v] + deg * CROWS1

    nc0, nc1, nc2 = _get_neffs(pads1, pads2)

    # ---- NEFF0: per-node u = x*dinv (fp16), dinv (f32) ----
    XO = np.zeros((N_CORES, NROWTOT), np.float32)
    XO[:, :NPC] = xf.reshape(N_CORES, NPC)
    DG = np.ones((N_CORES, NROWTOT), np.uint16)
    DG[:, :NPC] = degp1.reshape(N_CORES, NPC)
    in0 = [{"xo": XO[c].reshape(128, NROWPP),
            "dg": DG[c].reshape(128, NROWPP)} for c in range(N_CORES)]
    res0 = bass_utils.run_bass_kernel_spmd(nc0, in0,
                                           core_ids=list(range(N_CORES)))
    u_full = np.concatenate([
        np.asarray(res0.results[c]["uo"], np.float16).reshape(-1)[:NPC]
        for c in range(N_CORES)])
    dv_full = np.concatenate([
        np.asarray(res0.results[c]["dv"], np.float32).reshape(-1)[:NPC]
        for c in range(N_CORES)])

    # sorted-order dinv [8, 128, NROWPP]: value at (p, r) = rank r*128+p
    vs = dv_full[sorted_ids]
    arr = np.zeros((N_CORES, NROWTOT), np.float32)
    arr[:, :NPC] = vs
    DVO = np.ascontiguousarray(
        arr.reshape(N_CORES, NROWPP, 128).transpose(0, 2, 1))

    # ---- NEFF1: layer-1 grid of u[src] (+self), slot-major per chunk ----
    GU = np.zeros((N_CORES, 128 * totw1), np.uint16)
    u16v = u_full.view(np.uint16)
    GU[corei, flat1] = u16v[ssrc]
    GU[core_v, flat1_self] = u16v
    w1r = np.tile(W1.reshape(1, 4), (128, 1)).astype(np.float32)
    b1r = np.tile(b1.reshape(1, 4), (128, 1)).astype(np.float32)
    in1 = [{"gu": GU[c].view(np.float16).reshape(128, totw1),
            "dv": DVO[c], "w1r": w1r, "b1r": b1r}
           for c in range(N_CORES)]
    res1 = bass_utils.run_bass_kernel_spmd(nc1, in1,
                                           core_ids=list(range(N_CORES)))
    m_raw = [np.ascontiguousarray(
        np.asarray(res1.results[c]["mo"], np.float16).reshape(128, 4 * NROWPP))
        for c in range(N_CORES)]

    # M planes per node (global), channel-planar
    M_full = np.empty((4, N_NODES), np.float16)
    for c in range(N_CORES):
        for ch in range(4):
            plane = m_raw[c][:, ch * NROWPP:(ch + 1) * NROWPP]
            M_full[ch, sorted_ids[c]] = plane.T.reshape(-1)[:NPC]

    # ---- NEFF2: 4-channel grid of M[src] (+self), [s, c, r] per chunk ----
    GM = np.zeros((N_CORES, 128 * 4 * totw2), np.uint16)
    pbase = p_e * (4 * totw2) + roff2[r_e] + jslot * (4 * CROWS2)
    pbase_self = p_v * (4 * totw2) + roff2[r_v] + deg * (4 * CROWS2)
    for ch in range(4):
        m16v = M_full[ch].view(np.uint16)
        GM[corei, pbase + ch * CROWS2] = m16v[ssrc]
        GM[core_v, pbase_self + ch * CROWS2] = m16v
    w2r = np.tile(W2.reshape(1, 16), (128, 1)).astype(np.float32)
    b2r = np.tile(b2.reshape(1, 4), (128, 1)).astype(np.float32)
    in2 = [{"gm": GM[c].view(np.float16).reshape(128, 4 * totw2),
            "dv": DVO[c], "w2r": w2r, "b2r": b2r}
           for c in range(N_CORES)]
    res2 = bass_utils.run_bass_kernel_spmd(nc2, in2,
                                           core_ids=list(range(N_CORES)))

    out = np.empty((N_NODES, 4), np.float32)
    for c in range(N_CORES):
        O = np.asarray(res2.results[c]["out"], np.float32).reshape(
            128, 4, NROWPP)
        # O[p, j, r] -> rank s = r*128+p
        out[sorted_ids[c]] = O.transpose(2, 0, 1).reshape(NROWTOT, 4)[:NPC]
    return np.ascontiguousarray(out)
